# revision 1
# baseline (speedup 1.0000x reference)
"""BitLinearLRLS fused kernel for 8 Trainium2 NeuronCores.

Math (see reference):
    w_q       = clip(round(weight / 0.5), -1, 1)            # ternary, RNE ties
    x_mean    = mean(x, axis=(0,1))                         # [in]
    scale_eff = scale + lrls_A @ (lrls_B @ x_mean)          # [out]
    y         = x @ (w_q * scale_eff[:, None]).T

Key identity used: y = (x @ w_q.T) * scale_eff[None, :] — the big matmul
does not depend on scale_eff, so the data-dependent scale is applied as a
per-output-row epilogue on PSUM tiles.

Sharding: data-parallel over tokens. Each of the 8 cores takes 2048 tokens
(x transposed on host to [in, tokens] so the contraction dim lands on SBUF
partitions), streams the full quantized weight, and computes its y^T slice
[out, 2048]. The token-sum for x_mean is computed on-device per core and
AllReduce'd (16 KiB) across the 8 cores.

Quantization is exact vs the reference: w_q = 1*(w > 0.25) - 1*(w < -0.25),
which matches clip(round(2w with RNE), -1, 1) for all fp32 w including ties
(round(±0.5) -> 0 under RNE).

The matmul runs in float32r (TF32-like, 1 cycle/row at N>=512): weights in
{-1,0,1} are exact in fp32r; only x carries the ~2^-12 input rounding.
"""

import numpy as np

import concourse.bass as bass
import concourse.tile as tile
from concourse import bacc, mybir
from concourse.bass_utils import run_bass_kernel_spmd

F32 = mybir.dt.float32
F32R = mybir.dt.float32r


class Cfg:
    def __init__(self, tok=16384, din=4096, dout=4096, r=16,
                 tsh=None, oslab=1024, tblk=512, ncores=8):
        self.ncores = ncores
        self.tok = tok            # total tokens (B*S)
        self.din = din
        self.dout = dout
        self.r = r
        self.tsh = tsh or tok // ncores   # tokens per core
        self.oslab = oslab        # output features per resident W slab
        self.tblk = tblk          # moving-operand tile (tokens)
        self.kc = din // 128      # contraction chunks
        self.oc = dout // 128     # output chunks (scale_eff columns)
        self.nslab = dout // self.oslab
        self.ntblk = self.tsh // tblk
        self.nos = self.oslab // 128  # psum banks per t-block
        assert self.nos <= 8


def build(cfg: Cfg):
    nc = bacc.Bacc("TRN2", target_bir_lowering=False, debug=False,
                   enable_asserts=True, num_devices=cfg.ncores)

    xT = nc.dram_tensor("xT", [cfg.din, cfg.tsh], F32, kind="ExternalInput").ap()
    wT = nc.dram_tensor("wT", [cfg.din, cfg.dout], F32, kind="ExternalInput").ap()
    scale_pc = nc.dram_tensor("scale_pc", [128, cfg.oc], F32,
                              kind="ExternalInput").ap()
    at = nc.dram_tensor("at", [cfg.r, cfg.dout], F32, kind="ExternalInput").ap()
    btp = nc.dram_tensor("btp", [128, cfg.kc * cfg.r], F32,
                         kind="ExternalInput").ap()
    yT = nc.dram_tensor("yT", [cfg.dout, cfg.tsh], F32,
                        kind="ExternalOutput").ap()

    with tile.TileContext(nc) as tc:
        with tc.tile_pool(name="keep", bufs=1) as keep:
            sums = keep.tile([128, cfg.kc], F32)
            gsums = keep.tile([128, cfg.kc], F32)
            scale_eff = keep.tile([128, cfg.oc], F32)

            # ---- Phase 1: per-core token sums of x (free-axis reduce) ----
            with tc.tile_pool(name="p1", bufs=2) as p1:
                for ib in range(cfg.kc):
                    xs = p1.tile([128, cfg.tsh], F32, name="xs")
                    nc.sync.dma_start(out=xs, in_=xT[ib * 128:(ib + 1) * 128, :])
                    nc.vector.reduce_sum(out=sums[:, ib:ib + 1], in_=xs,
                                         axis=mybir.AxisListType.X)

            # ---- Phase 1b: AllReduce the partial sums across all cores ----
            with tc.tile_pool(name="cdram", bufs=1, space="DRAM") as cdram:
                cc_in = cdram.tile([128, cfg.kc], F32)
                cc_out = cdram.tile([128, cfg.kc], F32)
                nc.sync.dma_start(out=cc_in, in_=sums)
                nc.gpsimd.collective_compute(
                    "AllReduce",
                    mybir.AluOpType.add,
                    replica_groups=[list(range(cfg.ncores))],
                    ins=[cc_in.opt()],
                    outs=[cc_out.opt()],
                )
                nc.sync.dma_start(out=gsums, in_=cc_out)

            # ---- Phase 1c: scale_eff = scale + (A @ (B @ sum_x)) / tok ----
            with tc.tile_pool(name="lrls", bufs=1) as lp, \
                 tc.tile_pool(name="lrls_ps", bufs=2, space="PSUM") as lps:
                btp_sb = lp.tile([128, cfg.kc * cfg.r], F32)
                nc.sync.dma_start(out=btp_sb, in_=btp)
                at_sb = lp.tile([cfg.r, cfg.dout], F32)
                nc.sync.dma_start(out=at_sb, in_=at)
                sc_sb = lp.tile([128, cfg.oc], F32)
                nc.sync.dma_start(out=sc_sb, in_=scale_pc)

                psv = lps.tile([cfg.r, 1], F32, name="psv")
                for k in range(cfg.kc):
                    nc.tensor.matmul(
                        psv,
                        lhsT=btp_sb[:, k * cfg.r:(k + 1) * cfg.r],
                        rhs=gsums[:, k:k + 1],
                        start=(k == 0), stop=(k == cfg.kc - 1),
                    )
                vsb = lp.tile([cfg.r, 1], F32)
                nc.vector.tensor_copy(out=vsb, in_=psv)

                corr = lp.tile([128, cfg.oc], F32)
                for j in range(cfg.oc):
                    pcj = lps.tile([128, 1], F32, name="pcj")
                    nc.tensor.matmul(
                        pcj,
                        lhsT=at_sb[:, j * 128:(j + 1) * 128],
                        rhs=vsb,
                        start=True, stop=True,
                    )
                    nc.vector.tensor_copy(out=corr[:, j:j + 1], in_=pcj)

                nc.vector.tensor_scalar(
                    out=scale_eff, in0=corr,
                    scalar1=1.0 / cfg.tok, scalar2=None,
                    op0=mybir.AluOpType.mult,
                )
                nc.vector.tensor_tensor(
                    out=scale_eff, in0=scale_eff, in1=sc_sb,
                    op=mybir.AluOpType.add,
                )

            # ---- Phase 2: y^T = (w_q.T x)^T with epilogue scaling ----
            with tc.tile_pool(name="wq", bufs=1) as wqp, \
                 tc.tile_pool(name="wst", bufs=3) as wst, \
                 tc.tile_pool(name="qa", bufs=2) as qap, \
                 tc.tile_pool(name="qb", bufs=2) as qbp, \
                 tc.tile_pool(name="xst", bufs=4) as xst, \
                 tc.tile_pool(name="xr", bufs=4) as xrp, \
                 tc.tile_pool(name="yst", bufs=3) as yst, \
                 tc.tile_pool(name="ps", bufs=8, space="PSUM") as psp:
                for slab in range(cfg.nslab):
                    # load + ternary-quantize one weight slab, fp32r resident
                    wq_sb = wqp.tile([128, cfg.kc, cfg.oslab], F32R, name="wq_sb")
                    for ib in range(cfg.kc):
                        wr = wst.tile([128, cfg.oslab], F32, name="wr")
                        nc.sync.dma_start(
                            out=wr,
                            in_=wT[ib * 128:(ib + 1) * 128,
                                   slab * cfg.oslab:(slab + 1) * cfg.oslab])
                        ga = qap.tile([128, cfg.oslab], F32, name="ga")
                        nc.vector.tensor_scalar(
                            out=ga, in0=wr, scalar1=0.25, scalar2=None,
                            op0=mybir.AluOpType.is_gt)
                        gb = qbp.tile([128, cfg.oslab], F32, name="gb")
                        nc.gpsimd.tensor_scalar(
                            out=gb, in0=wr, scalar1=-0.25, scalar2=None,
                            op0=mybir.AluOpType.is_lt)
                        nc.vector.tensor_tensor(
                            out=wq_sb[:, ib, :], in0=ga, in1=gb,
                            op=mybir.AluOpType.subtract)

                    for tb in range(cfg.ntblk):
                        pst = [psp.tile([128, cfg.tblk], F32, name="pst")
                               for _ in range(cfg.nos)]
                        for i in range(cfg.kc):
                            xt = xst.tile([128, cfg.tblk], F32, name="xt")
                            nc.sync.dma_start(
                                out=xt,
                                in_=xT[i * 128:(i + 1) * 128,
                                       tb * cfg.tblk:(tb + 1) * cfg.tblk])
                            xr = xrp.tile([128, cfg.tblk], F32R, name="xr")
                            nc.vector.tensor_copy(out=xr, in_=xt)
                            for os_ in range(cfg.nos):
                                nc.tensor.matmul(
                                    pst[os_],
                                    lhsT=wq_sb[:, i, os_ * 128:(os_ + 1) * 128],
                                    rhs=xr,
                                    start=(i == 0), stop=(i == cfg.kc - 1),
                                )
                        for os_ in range(cfg.nos):
                            oi = slab * cfg.nos + os_
                            yt = yst.tile([128, cfg.tblk], F32, name="yt")
                            nc.scalar.activation(
                                out=yt, in_=pst[os_],
                                func=mybir.ActivationFunctionType.Copy,
                                scale=scale_eff[:, oi:oi + 1],
                            )
                            orow = slab * cfg.oslab + os_ * 128
                            nc.sync.dma_start(
                                out=yT[orow:orow + 128,
                                       tb * cfg.tblk:(tb + 1) * cfg.tblk],
                                in_=yt)

    nc.compile()
    return nc


def prep_inputs(cfg: Cfg, x, weight, scale, lrls_A, lrls_B):
    """Host-side sharding/layout marshalling (no arithmetic on the data)."""
    x_flat = np.ascontiguousarray(x.reshape(cfg.tok, cfg.din))
    xT_full = np.ascontiguousarray(x_flat.T)          # [din, tok]
    wT = np.ascontiguousarray(weight.T)               # [din, dout]
    at = np.ascontiguousarray(lrls_A.T)               # [r, dout]
    bt = np.ascontiguousarray(lrls_B.T)               # [din, r]
    btp = np.ascontiguousarray(
        bt.reshape(cfg.kc, 128, cfg.r).transpose(1, 0, 2).reshape(
            128, cfg.kc * cfg.r))
    scale_pc = np.ascontiguousarray(scale.reshape(cfg.oc, 128).T)

    in_maps = []
    for c in range(cfg.ncores):
        xT_c = np.ascontiguousarray(
            xT_full[:, c * cfg.tsh:(c + 1) * cfg.tsh])
        in_maps.append({"xT": xT_c, "wT": wT, "scale_pc": scale_pc,
                        "at": at, "btp": btp})
    return in_maps


def assemble_output(cfg: Cfg, results, out_shape):
    y_flat = np.empty((cfg.tok, cfg.dout), np.float32)
    for c in range(cfg.ncores):
        y_flat[c * cfg.tsh:(c + 1) * cfg.tsh, :] = results[c]["yT"].T
    return y_flat.reshape(out_shape)


_NC_CACHE = {}


def run(cfg: Cfg, x, weight, scale, lrls_A, lrls_B, out_shape, **run_kwargs):
    key = (cfg.tok, cfg.din, cfg.dout, cfg.tsh, cfg.oslab, cfg.tblk)
    if key not in _NC_CACHE:
        _NC_CACHE[key] = build(cfg)
    nc = _NC_CACHE[key]
    in_maps = prep_inputs(cfg, x, weight, scale, lrls_A, lrls_B)
    res = run_bass_kernel_spmd(nc, in_maps, core_ids=list(range(cfg.ncores)),
                               **run_kwargs)
    y = assemble_output(cfg, res.results, out_shape)
    return y, res


def kernel(x, weight, threshold, scale, lrls_A, lrls_B):
    # threshold input is unused: the reference hardcodes THRESH=0.5
    # (TrainState.threshold() at step 0), so the ternary cut sits at |w|=0.25.
    cfg = Cfg()
    x = np.asarray(x, np.float32)
    y, _ = run(cfg, x, np.asarray(weight, np.float32),
               np.asarray(scale, np.float32), np.asarray(lrls_A, np.float32),
               np.asarray(lrls_B, np.float32),
               out_shape=(x.shape[0], x.shape[1], np.asarray(weight).shape[0]))
    return y.astype(np.float32)


# revision 7
# speedup vs baseline: 2.0928x; 2.0928x over previous
"""BitLinearLRLS fused kernel for 8 Trainium2 NeuronCores.

Math (see reference):
    w_q       = clip(round(weight / 0.5), -1, 1)            # ternary, RNE ties
    x_mean    = mean(x, axis=(0,1))                         # [in]
    scale_eff = scale + lrls_A @ (lrls_B @ x_mean)          # [out]
    y         = x @ (w_q * scale_eff[:, None]).T

Key identity used: y = (x @ w_q.T) * scale_eff[None, :] — the big matmul
does not depend on scale_eff, so the data-dependent scale is applied as a
per-output-row epilogue on PSUM tiles.

Sharding: data-parallel over tokens. Each of the 8 cores takes 2048 tokens
(x transposed on host to [in, tokens] so the contraction dim lands on SBUF
partitions), streams the full quantized weight, and computes its y^T slice
[out, 2048]. The token-sum for x_mean is computed on-device per core and
AllReduce'd (16 KiB) across the 8 cores.

Quantization is exact vs the reference:
    w_q = int32_cast_rne(clamp(2w, -1.25, 1.25))
which matches clip(round(2w with RNE), -1, 1) for all fp32 w including ties
(the DVE float->int cast is round-to-nearest-even; verified on HW).
is_gt/is_lt ALU ops are avoided — they run ~20x slower than mult/max/min.

The matmul runs in float32r (TF32-like, 1 cycle/row at N>=512): weights in
{-1,0,1} are exact in fp32r; only x carries the ~2^-12 read rounding. x is
fed to the PE by declaring its DRAM tensor float32r and DMA-ing straight
into float32r tiles (bit layout is fp32-compatible; the PE rounds on read —
verified on HW to match a DVE-converted operand), so no per-tile cast pass.
"""

import numpy as np

import concourse.bass as bass
import concourse.tile as tile
from concourse import bacc, mybir
from concourse.bass_utils import run_bass_kernel_spmd

F32 = mybir.dt.float32
F32R = mybir.dt.float32r
I32 = mybir.dt.int32


class Cfg:
    def __init__(self, tok=16384, din=4096, dout=4096, r=16,
                 tsh=None, oslab=1024, tblk=512, ncores=8):
        self.ncores = ncores
        self.tok = tok            # total tokens (B*S)
        self.din = din
        self.dout = dout
        self.r = r
        self.tsh = tsh or tok // ncores   # tokens per core
        self.oslab = oslab        # output features per resident W slab
        self.tblk = tblk          # moving-operand tile (tokens)
        self.kc = din // 128      # contraction chunks
        self.oc = dout // 128     # output chunks (scale_eff columns)
        self.nslab = dout // self.oslab
        self.ntblk = self.tsh // tblk
        self.nos = self.oslab // 128  # psum banks per t-block
        assert self.nos <= 8


def build(cfg: Cfg):
    nc = bacc.Bacc("TRN2", target_bir_lowering=False, debug=False,
                   enable_asserts=True, num_devices=cfg.ncores)

    xT = nc.dram_tensor("xT", [cfg.din, cfg.tsh], F32R,
                        kind="ExternalInput").ap()
    wT = nc.dram_tensor("wT", [cfg.din, cfg.dout], F32, kind="ExternalInput").ap()
    scale_pc = nc.dram_tensor("scale_pc", [128, cfg.oc], F32,
                              kind="ExternalInput").ap()
    at = nc.dram_tensor("at", [cfg.r, cfg.dout], F32, kind="ExternalInput").ap()
    btp = nc.dram_tensor("btp", [128, cfg.kc * cfg.r], F32,
                         kind="ExternalInput").ap()
    yT = nc.dram_tensor("yT", [cfg.dout, cfg.tsh], F32,
                        kind="ExternalOutput").ap()

    with tile.TileContext(nc) as tc:
        with tc.tile_pool(name="keep", bufs=1) as keep:
            sums = keep.tile([128, cfg.kc], F32)
            gsums = keep.tile([128, cfg.kc], F32)
            scale_eff = keep.tile([128, cfg.oc], F32)

            # ---- Phase 1: per-core token sums of x (free-axis reduce) ----
            with tc.tile_pool(name="p1", bufs=2) as p1:
                for ib in range(cfg.kc):
                    xs = p1.tile([128, cfg.tsh], F32, name="xs")
                    nc.sync.dma_start(
                        out=xs,
                        in_=xT[ib * 128:(ib + 1) * 128, :].bitcast(F32))
                    nc.vector.reduce_sum(out=sums[:, ib:ib + 1], in_=xs,
                                         axis=mybir.AxisListType.X)

            # ---- Phase 1b: AllReduce the partial sums across all cores ----
            with tc.tile_pool(name="cdram", bufs=1, space="DRAM") as cdram:
                cc_in = cdram.tile([128, cfg.kc], F32)
                cc_out = cdram.tile([128, cfg.kc], F32)
                nc.sync.dma_start(out=cc_in, in_=sums)
                nc.gpsimd.collective_compute(
                    "AllReduce",
                    mybir.AluOpType.add,
                    replica_groups=[list(range(cfg.ncores))],
                    ins=[cc_in.opt()],
                    outs=[cc_out.opt()],
                )
                nc.sync.dma_start(out=gsums, in_=cc_out)

            # ---- Phase 1c: scale_eff = scale + (A @ (B @ sum_x)) / tok ----
            with tc.tile_pool(name="lrls", bufs=1) as lp, \
                 tc.tile_pool(name="lrls_ps", bufs=2, space="PSUM") as lps:
                btp_sb = lp.tile([128, cfg.kc * cfg.r], F32)
                nc.sync.dma_start(out=btp_sb, in_=btp)
                at_sb = lp.tile([cfg.r, cfg.dout], F32)
                nc.sync.dma_start(out=at_sb, in_=at)
                sc_sb = lp.tile([128, cfg.oc], F32)
                nc.sync.dma_start(out=sc_sb, in_=scale_pc)

                psv = lps.tile([cfg.r, 1], F32, name="psv")
                for k in range(cfg.kc):
                    nc.tensor.matmul(
                        psv,
                        lhsT=btp_sb[:, k * cfg.r:(k + 1) * cfg.r],
                        rhs=gsums[:, k:k + 1],
                        start=(k == 0), stop=(k == cfg.kc - 1),
                    )
                vsb = lp.tile([cfg.r, 1], F32)
                nc.vector.tensor_copy(out=vsb, in_=psv)

                corr = lp.tile([128, cfg.oc], F32)
                for j in range(cfg.oc):
                    pcj = lps.tile([128, 1], F32, name="pcj")
                    nc.tensor.matmul(
                        pcj,
                        lhsT=at_sb[:, j * 128:(j + 1) * 128],
                        rhs=vsb,
                        start=True, stop=True,
                    )
                    nc.vector.tensor_copy(out=corr[:, j:j + 1], in_=pcj)

                nc.vector.tensor_scalar(
                    out=scale_eff, in0=corr,
                    scalar1=1.0 / cfg.tok, scalar2=None,
                    op0=mybir.AluOpType.mult,
                )
                nc.vector.tensor_tensor(
                    out=scale_eff, in0=scale_eff, in1=sc_sb,
                    op=mybir.AluOpType.add,
                )

            # ---- Phase 2: y^T = (w_q.T x)^T with epilogue scaling ----
            with tc.tile_pool(name="wq", bufs=1) as wqp, \
                 tc.tile_pool(name="wst", bufs=3) as wst, \
                 tc.tile_pool(name="qa", bufs=2) as qap, \
                 tc.tile_pool(name="qb", bufs=2) as qbp, \
                 tc.tile_pool(name="xst", bufs=4) as xst, \
                 tc.tile_pool(name="yst", bufs=3) as yst, \
                 tc.tile_pool(name="ps", bufs=8, space="PSUM") as psp:
                for slab in range(cfg.nslab):
                    # load + ternary-quantize one weight slab, fp32r resident
                    # wq = int32_rne(clamp(2w, +-1.25)); int32 -> f32r on ACT
                    wq_sb = wqp.tile([128, cfg.kc, cfg.oslab], F32R, name="wq_sb")
                    for ib in range(cfg.kc):
                        wr = wst.tile([128, cfg.oslab], F32, name="wr")
                        nc.sync.dma_start(
                            out=wr,
                            in_=wT[ib * 128:(ib + 1) * 128,
                                   slab * cfg.oslab:(slab + 1) * cfg.oslab])
                        ga = qap.tile([128, cfg.oslab], F32, name="ga")
                        nc.vector.tensor_scalar(
                            out=ga, in0=wr, scalar1=2.0, scalar2=-1.25,
                            op0=mybir.AluOpType.mult, op1=mybir.AluOpType.max)
                        gb = qbp.tile([128, cfg.oslab], I32, name="gb")
                        nc.vector.tensor_scalar(
                            out=gb, in0=ga, scalar1=1.25, scalar2=None,
                            op0=mybir.AluOpType.min)
                        nc.scalar.activation(
                            out=wq_sb[:, ib, :], in_=gb,
                            func=mybir.ActivationFunctionType.Copy)

                    for tb in range(cfg.ntblk):
                        pst = [psp.tile([128, cfg.tblk], F32, name="pst")
                               for _ in range(cfg.nos)]
                        for i in range(cfg.kc):
                            xr = xst.tile([128, cfg.tblk], F32R, name="xr")
                            nc.sync.dma_start(
                                out=xr,
                                in_=xT[i * 128:(i + 1) * 128,
                                       tb * cfg.tblk:(tb + 1) * cfg.tblk])
                            for os_ in range(cfg.nos):
                                nc.tensor.matmul(
                                    pst[os_],
                                    lhsT=wq_sb[:, i, os_ * 128:(os_ + 1) * 128],
                                    rhs=xr,
                                    start=(i == 0), stop=(i == cfg.kc - 1),
                                )
                        for os_ in range(cfg.nos):
                            oi = slab * cfg.nos + os_
                            yt = yst.tile([128, cfg.tblk], F32, name="yt")
                            nc.scalar.activation(
                                out=yt, in_=pst[os_],
                                func=mybir.ActivationFunctionType.Copy,
                                scale=scale_eff[:, oi:oi + 1],
                            )
                            orow = slab * cfg.oslab + os_ * 128
                            nc.sync.dma_start(
                                out=yT[orow:orow + 128,
                                       tb * cfg.tblk:(tb + 1) * cfg.tblk],
                                in_=yt)

    nc.compile()
    return nc


def prep_inputs(cfg: Cfg, x, weight, scale, lrls_A, lrls_B):
    """Host-side sharding/layout marshalling (no arithmetic on the data)."""
    x_flat = np.ascontiguousarray(x.reshape(cfg.tok, cfg.din))
    xT_full = np.ascontiguousarray(x_flat.T)          # [din, tok]
    wT = np.ascontiguousarray(weight.T)               # [din, dout]
    at = np.ascontiguousarray(lrls_A.T)               # [r, dout]
    bt = np.ascontiguousarray(lrls_B.T)               # [din, r]
    btp = np.ascontiguousarray(
        bt.reshape(cfg.kc, 128, cfg.r).transpose(1, 0, 2).reshape(
            128, cfg.kc * cfg.r))
    scale_pc = np.ascontiguousarray(scale.reshape(cfg.oc, 128).T)

    in_maps = []
    for c in range(cfg.ncores):
        xT_c = np.ascontiguousarray(
            xT_full[:, c * cfg.tsh:(c + 1) * cfg.tsh])
        in_maps.append({"xT": xT_c, "wT": wT, "scale_pc": scale_pc,
                        "at": at, "btp": btp})
    return in_maps


def assemble_output(cfg: Cfg, results, out_shape):
    y_flat = np.empty((cfg.tok, cfg.dout), np.float32)
    for c in range(cfg.ncores):
        y_flat[c * cfg.tsh:(c + 1) * cfg.tsh, :] = results[c]["yT"].T
    return y_flat.reshape(out_shape)


_NC_CACHE = {}


def run(cfg: Cfg, x, weight, scale, lrls_A, lrls_B, out_shape, **run_kwargs):
    key = (cfg.tok, cfg.din, cfg.dout, cfg.tsh, cfg.oslab, cfg.tblk)
    if key not in _NC_CACHE:
        _NC_CACHE[key] = build(cfg)
    nc = _NC_CACHE[key]
    in_maps = prep_inputs(cfg, x, weight, scale, lrls_A, lrls_B)
    res = run_bass_kernel_spmd(nc, in_maps, core_ids=list(range(cfg.ncores)),
                               **run_kwargs)
    y = assemble_output(cfg, res.results, out_shape)
    return y, res


def kernel(x, weight, threshold, scale, lrls_A, lrls_B):
    # threshold input is unused: the reference hardcodes THRESH=0.5
    # (TrainState.threshold() at step 0), so the ternary cut sits at |w|=0.25.
    cfg = Cfg()
    x = np.asarray(x, np.float32)
    y, _ = run(cfg, x, np.asarray(weight, np.float32),
               np.asarray(scale, np.float32), np.asarray(lrls_A, np.float32),
               np.asarray(lrls_B, np.float32),
               out_shape=(x.shape[0], x.shape[1], np.asarray(weight).shape[0]))
    return y.astype(np.float32)


# revision 10
# speedup vs baseline: 2.3915x; 1.1427x over previous
"""BitLinearLRLS fused kernel for 8 Trainium2 NeuronCores.

Math (see reference):
    w_q       = clip(round(weight / 0.5), -1, 1)            # ternary, RNE ties
    x_mean    = mean(x, axis=(0,1))                         # [in]
    scale_eff = scale + lrls_A @ (lrls_B @ x_mean)          # [out]
    y         = x @ (w_q * scale_eff[:, None]).T

Key identity used: y = (x @ w_q.T) * scale_eff[None, :] — the big matmul
does not depend on scale_eff, so the data-dependent scale is applied as a
per-output-row epilogue on PSUM tiles.

Sharding: data-parallel over tokens. Each of the 8 cores takes 2048 tokens
(x transposed on host to [in, tokens] so the contraction dim lands on SBUF
partitions), streams the full quantized weight, and computes its y^T slice
[out, 2048]. The token-sum for x_mean is computed on-device per core and
AllReduce'd (16 KiB) across the 8 cores.

Quantization is exact vs the reference:
    w_q = int32_cast_rne(clamp(2w, -1.25, 1.25))
which matches clip(round(2w with RNE), -1, 1) for all fp32 w including ties
(the DVE float->int cast is round-to-nearest-even; verified on HW).
is_gt/is_lt ALU ops are avoided — they run ~20x slower than mult/max/min.

The matmul runs in float32r (TF32-like, 1 cycle/row at N>=512): weights in
{-1,0,1} are exact in fp32r; only x carries the ~2^-12 read rounding. x is
fed to the PE by declaring its DRAM tensor float32r and DMA-ing straight
into float32r tiles (bit layout is fp32-compatible; the PE rounds on read —
verified on HW to match a DVE-converted operand), so no per-tile cast pass.
"""

import numpy as np

import concourse.bass as bass
import concourse.tile as tile
from concourse import bacc, mybir
from concourse.bass_utils import run_bass_kernel_spmd

F32 = mybir.dt.float32
F32R = mybir.dt.float32r
I32 = mybir.dt.int32


class Cfg:
    def __init__(self, tok=16384, din=4096, dout=4096, r=16,
                 tsh=None, oslab=1024, tblk=512, ncores=8):
        self.ncores = ncores
        self.tok = tok            # total tokens (B*S)
        self.din = din
        self.dout = dout
        self.r = r
        self.tsh = tsh or tok // ncores   # tokens per core
        self.oslab = oslab        # output features per resident W slab
        self.tblk = tblk          # moving-operand tile (tokens)
        self.kc = din // 128      # contraction chunks
        self.oc = dout // 128     # output chunks (scale_eff columns)
        self.nslab = dout // self.oslab
        self.ntblk = self.tsh // tblk
        self.nos = self.oslab // 128  # psum banks per t-block
        assert self.nos <= 8


def build(cfg: Cfg):
    nc = bacc.Bacc("TRN2", target_bir_lowering=False, debug=False,
                   enable_asserts=True, num_devices=cfg.ncores)

    xT = nc.dram_tensor("xT", [cfg.din, cfg.tsh], F32R,
                        kind="ExternalInput").ap()
    wT = nc.dram_tensor("wT", [cfg.din, cfg.dout], F32, kind="ExternalInput").ap()
    scale_pc = nc.dram_tensor("scale_pc", [128, cfg.oc], F32,
                              kind="ExternalInput").ap()
    at = nc.dram_tensor("at", [cfg.r, cfg.dout], F32, kind="ExternalInput").ap()
    btp = nc.dram_tensor("btp", [128, cfg.kc * cfg.r], F32,
                         kind="ExternalInput").ap()
    yT = nc.dram_tensor("yT", [cfg.dout, cfg.tsh], F32,
                        kind="ExternalOutput").ap()

    with tile.TileContext(nc) as tc:
        with tc.tile_pool(name="keep", bufs=1) as keep:
            sums = keep.tile([128, cfg.kc], F32)
            gsums = keep.tile([128, cfg.kc], F32)
            scale_eff = keep.tile([128, cfg.oc], F32)

            # ---- Phase 1: per-core token sums of x (free-axis reduce) ----
            with tc.tile_pool(name="p1", bufs=2) as p1:
                for ib in range(cfg.kc):
                    xs = p1.tile([128, cfg.tsh], F32, name="xs")
                    nc.sync.dma_start(
                        out=xs,
                        in_=xT[ib * 128:(ib + 1) * 128, :].bitcast(F32))
                    nc.vector.reduce_sum(out=sums[:, ib:ib + 1], in_=xs,
                                         axis=mybir.AxisListType.X)

            # ---- Phase 1b: AllReduce the partial sums across all cores ----
            with tc.tile_pool(name="cdram", bufs=1, space="DRAM") as cdram:
                cc_in = cdram.tile([128, cfg.kc], F32)
                cc_out = cdram.tile([128, cfg.kc], F32)
                nc.sync.dma_start(out=cc_in, in_=sums)
                nc.gpsimd.collective_compute(
                    "AllReduce",
                    mybir.AluOpType.add,
                    replica_groups=[list(range(cfg.ncores))],
                    ins=[cc_in.opt()],
                    outs=[cc_out.opt()],
                )
                nc.sync.dma_start(out=gsums, in_=cc_out)

            # ---- Phase 1c: scale_eff = scale + (A @ (B @ sum_x)) / tok ----
            with tc.tile_pool(name="lrls", bufs=1) as lp, \
                 tc.tile_pool(name="lrls_ps", bufs=2, space="PSUM") as lps:
                btp_sb = lp.tile([128, cfg.kc * cfg.r], F32)
                nc.sync.dma_start(out=btp_sb, in_=btp)
                at_sb = lp.tile([cfg.r, cfg.dout], F32)
                nc.sync.dma_start(out=at_sb, in_=at)
                sc_sb = lp.tile([128, cfg.oc], F32)
                nc.sync.dma_start(out=sc_sb, in_=scale_pc)

                psv = lps.tile([cfg.r, 1], F32, name="psv")
                for k in range(cfg.kc):
                    nc.tensor.matmul(
                        psv,
                        lhsT=btp_sb[:, k * cfg.r:(k + 1) * cfg.r],
                        rhs=gsums[:, k:k + 1],
                        start=(k == 0), stop=(k == cfg.kc - 1),
                    )
                vsb = lp.tile([cfg.r, 1], F32)
                nc.vector.tensor_copy(out=vsb, in_=psv)

                corr = lp.tile([128, cfg.oc], F32)
                for j in range(cfg.oc):
                    pcj = lps.tile([128, 1], F32, name="pcj")
                    nc.tensor.matmul(
                        pcj,
                        lhsT=at_sb[:, j * 128:(j + 1) * 128],
                        rhs=vsb,
                        start=True, stop=True,
                    )
                    nc.vector.tensor_copy(out=corr[:, j:j + 1], in_=pcj)

                nc.vector.tensor_scalar(
                    out=scale_eff, in0=corr,
                    scalar1=1.0 / cfg.tok, scalar2=None,
                    op0=mybir.AluOpType.mult,
                )
                nc.vector.tensor_tensor(
                    out=scale_eff, in0=scale_eff, in1=sc_sb,
                    op=mybir.AluOpType.add,
                )

            # ---- Phase 2: y^T = (w_q.T x)^T with epilogue scaling ----
            with tc.tile_pool(name="wq", bufs=1) as wqp, \
                 tc.tile_pool(name="wst", bufs=3) as wst, \
                 tc.tile_pool(name="qa", bufs=2) as qap, \
                 tc.tile_pool(name="qb", bufs=2) as qbp, \
                 tc.tile_pool(name="xst", bufs=6) as xst, \
                 tc.tile_pool(name="yst", bufs=2) as yst, \
                 tc.tile_pool(name="ps", bufs=8, space="PSUM") as psp:
                for slab in range(cfg.nslab):
                    # load + ternary-quantize one weight slab, fp32r resident
                    # wq = int32_rne(clamp(2w, +-1.25)); int32 -> f32r on ACT
                    wq_sb = wqp.tile([128, cfg.kc, cfg.oslab], F32R, name="wq_sb")
                    for ib in range(cfg.kc):
                        wr = wst.tile([128, cfg.oslab], F32, name="wr")
                        # gpsimd SWDGE ring: keeps W stream off the x ring
                        nc.gpsimd.dma_start(
                            out=wr,
                            in_=wT[ib * 128:(ib + 1) * 128,
                                   slab * cfg.oslab:(slab + 1) * cfg.oslab])
                        ga = qap.tile([128, cfg.oslab], F32, name="ga")
                        nc.vector.tensor_scalar(
                            out=ga, in0=wr, scalar1=2.0, scalar2=-1.25,
                            op0=mybir.AluOpType.mult, op1=mybir.AluOpType.max)
                        gb = qbp.tile([128, cfg.oslab], I32, name="gb")
                        nc.vector.tensor_scalar(
                            out=gb, in0=ga, scalar1=1.25, scalar2=None,
                            op0=mybir.AluOpType.min)
                        nc.scalar.activation(
                            out=wq_sb[:, ib, :], in_=gb,
                            func=mybir.ActivationFunctionType.Copy)

                    for tb in range(cfg.ntblk):
                        pst = [psp.tile([128, cfg.tblk], F32, name="pst")
                               for _ in range(cfg.nos)]
                        for i in range(cfg.kc):
                            xr = xst.tile([128, cfg.tblk], F32R, name="xr")
                            nc.sync.dma_start(
                                out=xr,
                                in_=xT[i * 128:(i + 1) * 128,
                                       tb * cfg.tblk:(tb + 1) * cfg.tblk])
                            for os_ in range(cfg.nos):
                                nc.tensor.matmul(
                                    pst[os_],
                                    lhsT=wq_sb[:, i, os_ * 128:(os_ + 1) * 128],
                                    rhs=xr,
                                    start=(i == 0), stop=(i == cfg.kc - 1),
                                )
                        yt = yst.tile([128, cfg.nos, cfg.tblk], F32, name="yt")
                        for os_ in range(cfg.nos):
                            oi = slab * cfg.nos + os_
                            nc.scalar.activation(
                                out=yt[:, os_, :], in_=pst[os_],
                                func=mybir.ActivationFunctionType.Copy,
                                scale=scale_eff[:, oi:oi + 1],
                            )
                        # one batched store on the scalar HWDGE ring
                        nc.scalar.dma_start(
                            out=yT[slab * cfg.oslab:(slab + 1) * cfg.oslab,
                                   tb * cfg.tblk:(tb + 1) * cfg.tblk]
                            .rearrange("(c p) t -> p c t", p=128),
                            in_=yt)

    nc.compile()
    return nc


def prep_inputs(cfg: Cfg, x, weight, scale, lrls_A, lrls_B):
    """Host-side sharding/layout marshalling (no arithmetic on the data)."""
    x_flat = np.ascontiguousarray(x.reshape(cfg.tok, cfg.din))
    xT_full = np.ascontiguousarray(x_flat.T)          # [din, tok]
    wT = np.ascontiguousarray(weight.T)               # [din, dout]
    at = np.ascontiguousarray(lrls_A.T)               # [r, dout]
    bt = np.ascontiguousarray(lrls_B.T)               # [din, r]
    btp = np.ascontiguousarray(
        bt.reshape(cfg.kc, 128, cfg.r).transpose(1, 0, 2).reshape(
            128, cfg.kc * cfg.r))
    scale_pc = np.ascontiguousarray(scale.reshape(cfg.oc, 128).T)

    in_maps = []
    for c in range(cfg.ncores):
        xT_c = np.ascontiguousarray(
            xT_full[:, c * cfg.tsh:(c + 1) * cfg.tsh])
        in_maps.append({"xT": xT_c, "wT": wT, "scale_pc": scale_pc,
                        "at": at, "btp": btp})
    return in_maps


def assemble_output(cfg: Cfg, results, out_shape):
    y_flat = np.empty((cfg.tok, cfg.dout), np.float32)
    for c in range(cfg.ncores):
        y_flat[c * cfg.tsh:(c + 1) * cfg.tsh, :] = results[c]["yT"].T
    return y_flat.reshape(out_shape)


_NC_CACHE = {}


def run(cfg: Cfg, x, weight, scale, lrls_A, lrls_B, out_shape, **run_kwargs):
    key = (cfg.tok, cfg.din, cfg.dout, cfg.tsh, cfg.oslab, cfg.tblk)
    if key not in _NC_CACHE:
        _NC_CACHE[key] = build(cfg)
    nc = _NC_CACHE[key]
    in_maps = prep_inputs(cfg, x, weight, scale, lrls_A, lrls_B)
    res = run_bass_kernel_spmd(nc, in_maps, core_ids=list(range(cfg.ncores)),
                               **run_kwargs)
    y = assemble_output(cfg, res.results, out_shape)
    return y, res


def kernel(x, weight, threshold, scale, lrls_A, lrls_B):
    # threshold input is unused: the reference hardcodes THRESH=0.5
    # (TrainState.threshold() at step 0), so the ternary cut sits at |w|=0.25.
    cfg = Cfg()
    x = np.asarray(x, np.float32)
    y, _ = run(cfg, x, np.asarray(weight, np.float32),
               np.asarray(scale, np.float32), np.asarray(lrls_A, np.float32),
               np.asarray(lrls_B, np.float32),
               out_shape=(x.shape[0], x.shape[1], np.asarray(weight).shape[0]))
    return y.astype(np.float32)


# revision 15
# speedup vs baseline: 2.4295x; 1.0159x over previous
"""BitLinearLRLS fused kernel for 8 Trainium2 NeuronCores.

Math (see reference):
    w_q       = clip(round(weight / 0.5), -1, 1)            # ternary, RNE ties
    x_mean    = mean(x, axis=(0,1))                         # [in]
    scale_eff = scale + lrls_A @ (lrls_B @ x_mean)          # [out]
    y         = x @ (w_q * scale_eff[:, None]).T

Key identity used: y = (x @ w_q.T) * scale_eff[None, :] — the big matmul
does not depend on scale_eff, so the data-dependent scale is applied as a
per-output-row epilogue on PSUM tiles.

Sharding: data-parallel over tokens. Each of the 8 cores takes 2048 tokens
(x transposed on host to [in, tokens] so the contraction dim lands on SBUF
partitions), streams the full quantized weight, and computes its y^T slice
[out, 2048]. The token-sum for x_mean is computed on-device per core and
AllReduce'd (16 KiB) across the 8 cores.

Quantization is exact vs the reference:
    w_q = int32_cast_rne(clamp(2w, -1.25, 1.25))
which matches clip(round(2w with RNE), -1, 1) for all fp32 w including ties
(the DVE float->int cast is round-to-nearest-even; verified on HW).
is_gt/is_lt ALU ops are avoided — they run ~20x slower than mult/max/min.

The matmul runs in float32r (TF32-like, 1 cycle/row at N>=512): weights in
{-1,0,1} are exact in fp32r; only x carries the ~2^-12 read rounding. x is
fed to the PE by declaring its DRAM tensor float32r and DMA-ing straight
into float32r tiles (bit layout is fp32-compatible; the PE rounds on read —
verified on HW to match a DVE-converted operand), so no per-tile cast pass.
"""

import numpy as np

import concourse.bass as bass
import concourse.tile as tile
from concourse import bacc, mybir
from concourse.bass_utils import run_bass_kernel_spmd

F32 = mybir.dt.float32
F32R = mybir.dt.float32r
I32 = mybir.dt.int32


class Cfg:
    def __init__(self, tok=16384, din=4096, dout=4096, r=16,
                 tsh=None, oslab=1024, tblk=512, ncores=8):
        self.ncores = ncores
        self.tok = tok            # total tokens (B*S)
        self.din = din
        self.dout = dout
        self.r = r
        self.tsh = tsh or tok // ncores   # tokens per core
        self.oslab = oslab        # output features per resident W slab
        self.tblk = tblk          # moving-operand tile (tokens)
        self.kc = din // 128      # contraction chunks
        self.oc = dout // 128     # output chunks (scale_eff columns)
        self.nslab = dout // self.oslab
        self.ntblk = self.tsh // tblk
        self.nos = self.oslab // 128  # psum banks per t-block
        assert self.nos <= 8


def build(cfg: Cfg):
    nc = bacc.Bacc("TRN2", target_bir_lowering=False, debug=False,
                   enable_asserts=True, num_devices=cfg.ncores)

    xT = nc.dram_tensor("xT", [cfg.din, cfg.tsh], F32R,
                        kind="ExternalInput").ap()
    wT = nc.dram_tensor("wT", [cfg.din, cfg.dout], F32, kind="ExternalInput").ap()
    scale_pc = nc.dram_tensor("scale_pc", [128, cfg.oc], F32,
                              kind="ExternalInput").ap()
    at = nc.dram_tensor("at", [cfg.r, cfg.dout], F32, kind="ExternalInput").ap()
    btp = nc.dram_tensor("btp", [128, cfg.kc * cfg.r], F32,
                         kind="ExternalInput").ap()
    yT = nc.dram_tensor("yT", [cfg.dout, cfg.tsh], F32,
                        kind="ExternalOutput").ap()

    with tile.TileContext(nc) as tc:
        with tc.tile_pool(name="keep", bufs=1) as keep:
            sums = keep.tile([128, cfg.kc], F32)
            gsums = keep.tile([128, cfg.kc], F32)
            scale_eff = keep.tile([128, cfg.oc], F32)

            # ---- Phase 1: per-core token sums of x (free-axis reduce) ----
            # pass-1 loads ride the scalar HWDGE ring: the sync ring must
            # start feeding matmul x-tiles at t=0, and the scalar ring's
            # first y store isn't due until ~250us in.
            with tc.tile_pool(name="p1", bufs=2) as p1:
                for ib in range(cfg.kc):
                    xs = p1.tile([128, cfg.tsh], F32, name="xs")
                    nc.scalar.dma_start(
                        out=xs,
                        in_=xT[ib * 128:(ib + 1) * 128, :].bitcast(F32))
                    nc.vector.reduce_sum(out=sums[:, ib:ib + 1], in_=xs,
                                         axis=mybir.AxisListType.X)

            # ---- Phase 1b: AllReduce the partial sums across all cores ----
            with tc.tile_pool(name="cdram", bufs=1, space="DRAM") as cdram:
                cc_in = cdram.tile([128, cfg.kc], F32)
                cc_out = cdram.tile([128, cfg.kc], F32)
                nc.scalar.dma_start(out=cc_in, in_=sums)
                nc.gpsimd.collective_compute(
                    "AllReduce",
                    mybir.AluOpType.add,
                    replica_groups=[list(range(cfg.ncores))],
                    ins=[cc_in.opt()],
                    outs=[cc_out.opt()],
                )
                nc.scalar.dma_start(out=gsums, in_=cc_out)

            # ---- Phase 1c: scale_eff = scale + (A @ (B @ sum_x)) / tok ----
            with tc.tile_pool(name="lrls", bufs=1) as lp, \
                 tc.tile_pool(name="lrls_ps", bufs=2, space="PSUM") as lps:
                btp_sb = lp.tile([128, cfg.kc * cfg.r], F32)
                nc.scalar.dma_start(out=btp_sb, in_=btp)
                at_sb = lp.tile([cfg.r, cfg.dout], F32)
                nc.scalar.dma_start(out=at_sb, in_=at)
                sc_sb = lp.tile([128, cfg.oc], F32)
                nc.scalar.dma_start(out=sc_sb, in_=scale_pc)

                psv = lps.tile([cfg.r, 1], F32, name="psv")
                for k in range(cfg.kc):
                    nc.tensor.matmul(
                        psv,
                        lhsT=btp_sb[:, k * cfg.r:(k + 1) * cfg.r],
                        rhs=gsums[:, k:k + 1],
                        start=(k == 0), stop=(k == cfg.kc - 1),
                    )
                vsb = lp.tile([cfg.r, 1], F32)
                nc.vector.tensor_copy(out=vsb, in_=psv)

                corr = lp.tile([128, cfg.oc], F32)
                for j in range(cfg.oc):
                    pcj = lps.tile([128, 1], F32, name="pcj")
                    nc.tensor.matmul(
                        pcj,
                        lhsT=at_sb[:, j * 128:(j + 1) * 128],
                        rhs=vsb,
                        start=True, stop=True,
                    )
                    nc.vector.tensor_copy(out=corr[:, j:j + 1], in_=pcj)

                nc.vector.tensor_scalar(
                    out=scale_eff, in0=corr,
                    scalar1=1.0 / cfg.tok, scalar2=None,
                    op0=mybir.AluOpType.mult,
                )
                nc.vector.tensor_tensor(
                    out=scale_eff, in0=scale_eff, in1=sc_sb,
                    op=mybir.AluOpType.add,
                )

            # ---- Phase 2: y^T = (w_q.T x)^T with epilogue scaling ----
            with tc.tile_pool(name="wq", bufs=1) as wqp, \
                 tc.tile_pool(name="wst", bufs=3) as wst, \
                 tc.tile_pool(name="qa", bufs=2) as qap, \
                 tc.tile_pool(name="qb", bufs=2) as qbp, \
                 tc.tile_pool(name="xst", bufs=8) as xst, \
                 tc.tile_pool(name="yst", bufs=2) as yst, \
                 tc.tile_pool(name="ps", bufs=8, space="PSUM") as psp:
                for slab in range(cfg.nslab):
                    # load + ternary-quantize one weight slab, fp32r resident
                    # wq = int32_rne(clamp(2w, +-1.25)); int32 -> f32r on ACT
                    wq_sb = wqp.tile([128, cfg.kc, cfg.oslab], F32R, name="wq_sb")
                    for ib in range(cfg.kc):
                        wr = wst.tile([128, cfg.oslab], F32, name="wr")
                        # gpsimd SWDGE ring: keeps W stream off the x ring
                        nc.gpsimd.dma_start(
                            out=wr,
                            in_=wT[ib * 128:(ib + 1) * 128,
                                   slab * cfg.oslab:(slab + 1) * cfg.oslab])
                        ga = qap.tile([128, cfg.oslab], F32, name="ga")
                        nc.vector.tensor_scalar(
                            out=ga, in0=wr, scalar1=2.0, scalar2=-1.25,
                            op0=mybir.AluOpType.mult, op1=mybir.AluOpType.max)
                        gb = qbp.tile([128, cfg.oslab], I32, name="gb")
                        nc.vector.tensor_scalar(
                            out=gb, in0=ga, scalar1=1.25, scalar2=None,
                            op0=mybir.AluOpType.min)
                        nc.scalar.activation(
                            out=wq_sb[:, ib, :], in_=gb,
                            func=mybir.ActivationFunctionType.Copy)

                    for tb in range(cfg.ntblk):
                        pst = [psp.tile([128, cfg.tblk], F32, name="pst")
                               for _ in range(cfg.nos)]
                        for i in range(cfg.kc):
                            xr = xst.tile([128, cfg.tblk], F32R, name="xr")
                            nc.sync.dma_start(
                                out=xr,
                                in_=xT[i * 128:(i + 1) * 128,
                                       tb * cfg.tblk:(tb + 1) * cfg.tblk])
                            for os_ in range(cfg.nos):
                                nc.tensor.matmul(
                                    pst[os_],
                                    lhsT=wq_sb[:, i, os_ * 128:(os_ + 1) * 128],
                                    rhs=xr,
                                    start=(i == 0), stop=(i == cfg.kc - 1),
                                )
                        yt = yst.tile([128, cfg.nos, cfg.tblk], F32, name="yt")
                        for os_ in range(cfg.nos):
                            oi = slab * cfg.nos + os_
                            nc.scalar.activation(
                                out=yt[:, os_, :], in_=pst[os_],
                                func=mybir.ActivationFunctionType.Copy,
                                scale=scale_eff[:, oi:oi + 1],
                            )
                        # one batched store on the scalar HWDGE ring
                        nc.scalar.dma_start(
                            out=yT[slab * cfg.oslab:(slab + 1) * cfg.oslab,
                                   tb * cfg.tblk:(tb + 1) * cfg.tblk]
                            .rearrange("(c p) t -> p c t", p=128),
                            in_=yt)

    nc.compile()
    return nc


def prep_inputs(cfg: Cfg, x, weight, scale, lrls_A, lrls_B):
    """Host-side sharding/layout marshalling (no arithmetic on the data)."""
    x_flat = np.ascontiguousarray(x.reshape(cfg.tok, cfg.din))
    xT_full = np.ascontiguousarray(x_flat.T)          # [din, tok]
    wT = np.ascontiguousarray(weight.T)               # [din, dout]
    at = np.ascontiguousarray(lrls_A.T)               # [r, dout]
    bt = np.ascontiguousarray(lrls_B.T)               # [din, r]
    btp = np.ascontiguousarray(
        bt.reshape(cfg.kc, 128, cfg.r).transpose(1, 0, 2).reshape(
            128, cfg.kc * cfg.r))
    scale_pc = np.ascontiguousarray(scale.reshape(cfg.oc, 128).T)

    in_maps = []
    for c in range(cfg.ncores):
        xT_c = np.ascontiguousarray(
            xT_full[:, c * cfg.tsh:(c + 1) * cfg.tsh])
        in_maps.append({"xT": xT_c, "wT": wT, "scale_pc": scale_pc,
                        "at": at, "btp": btp})
    return in_maps


def assemble_output(cfg: Cfg, results, out_shape):
    y_flat = np.empty((cfg.tok, cfg.dout), np.float32)
    for c in range(cfg.ncores):
        y_flat[c * cfg.tsh:(c + 1) * cfg.tsh, :] = results[c]["yT"].T
    return y_flat.reshape(out_shape)


_NC_CACHE = {}


def run(cfg: Cfg, x, weight, scale, lrls_A, lrls_B, out_shape, **run_kwargs):
    key = (cfg.tok, cfg.din, cfg.dout, cfg.tsh, cfg.oslab, cfg.tblk)
    if key not in _NC_CACHE:
        _NC_CACHE[key] = build(cfg)
    nc = _NC_CACHE[key]
    in_maps = prep_inputs(cfg, x, weight, scale, lrls_A, lrls_B)
    res = run_bass_kernel_spmd(nc, in_maps, core_ids=list(range(cfg.ncores)),
                               **run_kwargs)
    y = assemble_output(cfg, res.results, out_shape)
    return y, res


def kernel(x, weight, threshold, scale, lrls_A, lrls_B):
    # threshold input is unused: the reference hardcodes THRESH=0.5
    # (TrainState.threshold() at step 0), so the ternary cut sits at |w|=0.25.
    cfg = Cfg()
    x = np.asarray(x, np.float32)
    y, _ = run(cfg, x, np.asarray(weight, np.float32),
               np.asarray(scale, np.float32), np.asarray(lrls_A, np.float32),
               np.asarray(lrls_B, np.float32),
               out_shape=(x.shape[0], x.shape[1], np.asarray(weight).shape[0]))
    return y.astype(np.float32)
